# revision 15
# baseline (speedup 1.0000x reference)
"""Trainium2 Bass kernel for CGAtNet-style GNN message passing.

Strategy (8 NeuronCores, SPMD, no collectives needed):
  - Host: assign the 10000 nodes to 80 windows of <=128 nodes (LPT-balanced by
    in-degree), 10 windows per core.  Edges are grouped by the window of their
    dst node and padded to a fixed per-window edge count E_WIN, so every core
    runs the identical compiled graph and produces output rows for a disjoint
    node set (no cross-core reduction at all).
  - Device, per window:
      * dma_gather (transposed) pulls x[dst]^T and x[src]^T columns in bf16
      * edge-MLP runs in transposed (feature-major) space on TensorE
      * M-net layer-1 per 128-edge subtile -> g_m [128e, 512] (edge-major)
      * A-net layer-1 feature-major per head + alpha = wa2 . lrelu(h1) on PE,
        exp on ScalarE, tiny PE transposes give per-edge softmax weights
      * per-node aggregation of ex-weighted hidden activations via an
        iota==dst indicator matmul accumulated in PSUM over the window
      * window epilogue applies M-net layer-2 per *node* (32x fewer FLOPs than
        per edge), normalizes by the softmax denominator, means over heads.
  - Softmax max-subtraction is dropped: softmax is shift-invariant and alpha
    here is O(1), so exp() is numerically safe.

Self-contained: hardcodes all shapes from the problem spec.
"""

import sys

for _p in ("/opt/trn_rl_repo",):
    if _p not in sys.path:
        sys.path.append(_p)

from contextlib import ExitStack

import ml_dtypes
import numpy as np

import concourse.bacc as bacc
import concourse.bass as bass
import concourse.mybir as mybir
import concourse.tile as tile
from concourse.bass_utils import run_bass_kernel_spmd
from concourse.library_config import mlp as _mlp_lib

BF16 = ml_dtypes.bfloat16

N_NODES = 10000
N_EDGES = 320000
C = 64
H = 4
HID = 128
N_CORES = 8
WPC = 10                 # windows per core
NW = N_CORES * WPC       # 80 windows
WIN = 128                # nodes per window

AF = mybir.ActivationFunctionType
ALU = mybir.AluOpType
DT = mybir.dt

TRACE = False            # set True by test.py to capture a profile
LAST_RESULT = {}         # filled with exec_time_ns etc. for test.py


# --------------------------------------------------------------------------
# host-side preparation
# --------------------------------------------------------------------------

def _assign_windows(deg):
    """LPT-balance nodes into NW windows of <=WIN nodes each."""
    import heapq

    order = np.argsort(-deg, kind="stable")
    heap = [(0, wi) for wi in range(NW)]
    heapq.heapify(heap)
    nnodes = np.zeros(NW, np.int64)
    ecount = np.zeros(NW, np.int64)
    win_of = np.empty(N_NODES, np.int64)
    slot_of = np.empty(N_NODES, np.int64)
    for n in order:
        while True:
            e, wi = heapq.heappop(heap)
            if nnodes[wi] < WIN:
                break
        win_of[n] = wi
        slot_of[n] = nnodes[wi]
        nnodes[wi] += 1
        ecount[wi] += deg[n]
        if nnodes[wi] < WIN:
            heapq.heappush(heap, (int(ecount[wi]), wi))
    return win_of, slot_of, ecount


def _wrap_idx(a):
    """dma_gather(transpose=True) emits out col c = in[idxw[c%16, c//16]];
    column-major wrap makes output columns follow logical index order."""
    return np.tile(a.reshape(-1, 16).T, (8, 1))


def _prep(x, edge_index, edge_attr, We1, be1, We2, be2,
          Wa1, ba1, Wa2, ba2, Wm1, bm1, Wm2, bm2):
    src = np.asarray(edge_index[0], np.int64)
    dst = np.asarray(edge_index[1], np.int64)
    x = np.asarray(x, np.float32)
    edge_attr = np.asarray(edge_attr, np.float32)

    deg = np.bincount(dst, minlength=N_NODES)
    win_of, slot_of, ecount = _assign_windows(deg)
    e_win = int(ecount.max())
    e_win = ((e_win + 127) // 128) * 128
    e_win = max(e_win, 512)

    w_of_edge = win_of[dst]
    eorder = np.argsort(w_of_edge, kind="stable")
    wcounts = np.bincount(w_of_edge, minlength=NW)
    woff = np.concatenate([[0], np.cumsum(wcounts)])

    src_p = np.zeros((NW, e_win), np.int16)
    dst_p = np.zeros((NW, e_win), np.int16)
    dw_p = np.full((NW, e_win), 255.0, np.float32)
    ea_p = np.zeros((NW, e_win, C), np.float32)
    for wi in range(NW):
        sl = eorder[woff[wi]:woff[wi + 1]]
        c = len(sl)
        src_p[wi, :c] = src[sl]
        dst_p[wi, :c] = dst[sl]
        dw_p[wi, :c] = slot_of[dst[sl]]
        ea_p[wi, :c] = edge_attr[sl]

    nsub = e_win // 128
    icols = e_win // 16

    # window-node id table (slot order), padded with 0
    node_tab = np.zeros((NW, 128), np.int16)
    node_tab[win_of, slot_of] = np.arange(N_NODES, dtype=np.int16)

    ar128 = np.arange(128, dtype=np.float32)
    per_core = []
    for cidx in range(N_CORES):
        ws = range(WPC * cidx, WPC * cidx + WPC)
        eaT = np.concatenate([ea_p[w].T for w in ws], axis=1).astype(BF16)
        gidx = np.concatenate(
            [blk for w in ws for blk in (_wrap_idx(src_p[w]), _wrap_idx(node_tab[w]))],
            axis=1)
        indt_blocks = []
        indn_blocks = []
        for w in ws:
            dw = dw_p[w].reshape(nsub, 128)
            indt_blocks.append(
                (dw[:, :, None] == ar128).transpose(1, 0, 2).reshape(128, nsub * 128))
            indn_blocks.append(ar128[:, None] == dw.reshape(1, nsub * 128))
        indt = np.concatenate(indt_blocks, axis=1).astype(BF16)
        indn = np.concatenate(indn_blocks, axis=1).astype(BF16)
        per_core.append({"eaT": eaT, "gidx": gidx, "indt": indt, "indn": indn})

    xpad = np.zeros((N_NODES, 128), BF16)
    xpad[:, :C] = x.astype(BF16)

    # weights / constants (replicated across cores)
    Wa1 = np.asarray(Wa1, np.float32)
    Wm1 = np.asarray(Wm1, np.float32)
    assert np.allclose(np.asarray(ba1), 0) and np.allclose(np.asarray(bm1), 0), \
        "nonzero layer-1 MH biases unsupported by this kernel build"
    consts = {
        "xpad": xpad,
        "wtopm": Wm1[:, :, 0:128].transpose(2, 0, 1).reshape(128, 512).astype(BF16),
        "wbotm": Wm1[:, :, 128:192].transpose(2, 0, 1).reshape(64, 512).astype(BF16),
        "wa1t": Wa1[:, :, 0:128].transpose(2, 0, 1).reshape(128, 512).astype(BF16),
        "wa1b": Wa1[:, :, 128:192].transpose(2, 0, 1).reshape(64, 512).astype(BF16),
        "wa2r": np.repeat(np.asarray(Wa2, np.float32)[:, 0, :].T, 32, axis=1).astype(BF16).copy(),
        "wm2t": np.asarray(Wm2, np.float32).transpose(2, 0, 1).reshape(128, 256).astype(BF16),
        "we1t": np.asarray(We1, np.float32).T.astype(BF16).copy(),
        "we2t": np.asarray(We2, np.float32).T.astype(BF16).copy(),
        "ident": np.eye(128, dtype=np.float32).astype(BF16),
        "be1c": np.asarray(be1, np.float32).reshape(64, 1).copy(),
        "bxe": np.concatenate([np.zeros(64, np.float32),
                               np.asarray(be2, np.float32).reshape(64)]).reshape(128, 1).copy(),
        "ba2c": _ba2col(ba2),
        "bm2m": np.tile(np.asarray(bm2, np.float32).mean(axis=0), (128, 1)).copy(),
    }

    meta = {"e_win": e_win, "nsub": nsub, "icols": icols,
            "win_of": win_of, "slot_of": slot_of, "deg": deg}
    return per_core, consts, meta


def _ba2col(ba2):
    v = np.asarray(ba2, np.float32).reshape(H)
    return np.repeat(v, 32).reshape(128, 1).copy()


# --------------------------------------------------------------------------
# device graph
# --------------------------------------------------------------------------

def _build(e_win):
    nsub = e_win // 128
    icols = e_win // 16
    nc = bacc.Bacc("TRN2", target_bir_lowering=False)
    bf, f32, i16 = DT.bfloat16, DT.float32, DT.int16

    P = lambda name, shape, dt: nc.declare_dram_parameter(name, shape, dt, isOutput=False)
    xpad = P("xpad", [N_NODES, 128], bf)
    eaT = P("eaT", [64, WPC * e_win], bf)
    gidx = P("gidx", [128, WPC * (icols + 8)], i16)
    indt = P("indt", [128, WPC * nsub * 128], bf)
    indn = P("indn", [128, WPC * nsub * 128], bf)
    wtopm = P("wtopm", [128, 512], bf)
    wbotm = P("wbotm", [64, 512], bf)
    wa1t = P("wa1t", [128, 512], bf)
    wa1b = P("wa1b", [64, 512], bf)
    wa2r = P("wa2r", [128, 128], bf)
    wm2t = P("wm2t", [128, 256], bf)
    we1t = P("we1t", [64, 64], bf)
    we2t = P("we2t", [64, 64], bf)
    ident = P("ident", [128, 128], bf)
    be1c = P("be1c", [64, 1], f32)
    bxe = P("bxe", [128, 1], f32)
    ba2c = P("ba2c", [128, 1], f32)
    bm2m = P("bm2m", [128, 64], f32)
    out = nc.declare_dram_parameter("out", [WPC * 128, 64], f32, isOutput=True)

    with tile.TileContext(nc) as tc, ExitStack() as ctx:
        pool = lambda name, bufs, **kw: ctx.enter_context(
            tc.tile_pool(name=name, bufs=bufs, **kw))
        cpool = pool("consts", 1)
        gath = pool("gath", 6)
        eap = pool("eap", 2)
        ip = pool("ip", 2)
        wk = pool("work", 2)
        wk3 = pool("work3", 3)
        ps_h1 = pool("ps_h1", 2, space="PSUM")
        ps_ax = pool("ps_ax", 2, space="PSUM")
        ps_g = pool("ps_g", 1, space="PSUM")
        ps_s = pool("ps_s", 1, space="PSUM")
        ps_m = pool("ps_m", 2, space="PSUM")

        def cload(name, src_ap, shape, dt):
            t = cpool.tile(shape, dt, tag=name)
            nc.sync.dma_start(t[:], src_ap)
            return t

        c_wtopm = cload("wtopm", wtopm[:], [128, 512], bf)
        c_wbotm = cload("wbotm", wbotm[:], [64, 512], bf)
        c_wa1t = cload("wa1t", wa1t[:], [128, 512], bf)
        c_wa1b = cload("wa1b", wa1b[:], [64, 512], bf)
        c_wa2r = cload("wa2r", wa2r[:], [128, 128], bf)
        c_wm2t = cload("wm2t", wm2t[:], [128, 256], bf)
        c_we1t = cload("we1t", we1t[:], [64, 64], bf)
        c_we2t = cload("we2t", we2t[:], [64, 64], bf)
        c_ident = cload("ident", ident[:], [128, 128], bf)
        c_be1c = cload("be1c", be1c[:], [64, 1], f32)
        c_bxe = cload("bxe", bxe[:], [128, 1], f32)
        c_ba2c = cload("ba2c", ba2c[:], [128, 1], f32)
        c_bm2m = cload("bm2m", bm2m[:], [128, 64], f32)
        c_idx = cload("gidx", gidx[:], [128, WPC * (icols + 8)], i16)

        nc.gpsimd.load_library(_mlp_lib)

        for w in range(WPC):
            xi = gath.tile([128, 1, e_win], bf, tag="m12")
            xj = gath.tile([128, 1, e_win], bf, tag="gath")
            xw = gath.tile([128, 1, 128], bf, tag="xw")
            ib = w * (icols + 8)
            nc.gpsimd.dma_gather(xj[:], xpad[:], c_idx[:, ib:ib + icols],
                                 e_win, e_win, 128, transpose=True,
                                 single_packet=False)
            nc.gpsimd.dma_gather(xw[:], xpad[:], c_idx[:, ib + icols:ib + icols + 8],
                                 128, 128, 128, transpose=False)
            ea = eap.tile([64, e_win], bf)
            nc.sync.dma_start(ea[:], eaT[:, w * e_win:(w + 1) * e_win])
            tind = ip.tile([128, nsub * 128], bf, tag="indt")
            nc.sync.dma_start(tind[:], indt[:, w * nsub * 128:(w + 1) * nsub * 128])
            nind = ip.tile([128, nsub * 128], bf, tag="indn")
            nc.sync.dma_start(nind[:], indn[:, w * nsub * 128:(w + 1) * nsub * 128])

            # edge-MLP (feature-major) + selector-gathered x_i share one psum
            for c0 in range(0, e_win, 512):
                cw = min(512, e_win - c0)
                pe1 = ps_m.tile([64, 512], f32, tag="misc")
                nc.tensor.matmul(pe1[:, :cw], c_we1t[:], ea[:, c0:c0 + cw])
                te = wk.tile([64, 512], bf, tag="te")
                nc.scalar.activation(te[:, :cw], pe1[:, :cw], AF.Prelu,
                                     bias=c_be1c[:], alpha=0.01)
                xie = ps_m.tile([128, 512], f32, tag="misc")
                for sl in range(cw // 128):
                    nc.tensor.matmul(xie[0:64, 128 * sl:128 * sl + 128],
                                     xw[:, 0, 0:64],
                                     nind[:, c0 + 128 * sl:c0 + 128 * sl + 128],
                                     tile_position=(0, 0))
                nc.tensor.matmul(xie[64:128, :cw], c_we2t[:], te[:, :cw],
                                 tile_position=(0, 64))
                nc.scalar.activation(xi[0:128, 0, c0:c0 + cw], xie[:, :cw],
                                     AF.Identity, bias=c_bxe[:])

            G_ps = ps_g.tile([128, 512], f32)
            S_ps = ps_s.tile([128, 4], f32)
            sub = 0
            for g0 in range(0, e_win, 512):
                fg = min(512, e_win - g0)
                nsg = fg // 128
                # A-net (feature-major) + alpha on PE
                aps = ps_ax.tile([128, 512], f32, tag="ax")
                for h in range(H):
                    gaps = ps_h1.tile([128, 512], f32, tag="h1")
                    nc.tensor.matmul(gaps[:, :fg], c_wa1t[:, 128 * h:128 * h + 128],
                                     xi[0:128, 0, g0:g0 + fg], start=True, stop=False)
                    nc.tensor.matmul(gaps[:, :fg], c_wa1b[:, 128 * h:128 * h + 128],
                                     xj[0:64, 0, g0:g0 + fg], start=False, stop=True)
                    ga = wk.tile([128, 512], bf, tag="ga")
                    nc.scalar.activation(ga[:, :fg], gaps[:, :fg], AF.Prelu, alpha=0.01)
                    nc.tensor.matmul(aps[32 * h:32 * h + 32, :fg],
                                     c_wa2r[:, 32 * h:32 * h + 32], ga[:, :fg],
                                     tile_position=(0, 32 * h))
                # row 32h..32h+32 of aps all hold head h's alpha (wa2 replicated)
                exsb = wk.tile([128, 512], bf, tag="ex")
                nc.scalar.activation(exsb[:, :fg], aps[:, :fg], AF.Exp,
                                     bias=c_ba2c[:])
                extp = ps_ax.tile([128, 4, 128], bf, tag="ax")
                for sl in range(nsg):
                    nc.tensor.transpose(extp[:, sl, :],
                                        exsb[:, 128 * sl:128 * sl + 128],
                                        c_ident[:])
                exts = wk.tile([128, 4, 4], bf, tag="exts")
                nc.vector.tensor_copy(exts[:], extp[:, :, 0:128:32])

                # M-net subtiles + indicator aggregation
                for sl in range(nsg):
                    e0 = g0 + 128 * sl
                    h1m = ps_h1.tile([128, 512], f32, tag="h1")
                    nc.tensor.matmul(h1m[:], xi[0:128, 0, e0:e0 + 128], c_wtopm[:],
                                     start=True, stop=False)
                    nc.tensor.matmul(h1m[:], xj[0:64, 0, e0:e0 + 128], c_wbotm[:],
                                     start=False, stop=True)
                    gm = wk3.tile([128, 512], bf, tag="gm")
                    nc.scalar.activation(gm[:], h1m[:], AF.Prelu, alpha=0.01)
                    gme = wk3.tile([128, 512], bf, tag="gme")
                    nc.vector.tensor_mul(
                        gme[:].rearrange("p (h j) -> p h j", h=H),
                        gm[:].rearrange("p (h j) -> p h j", h=H),
                        exts[:, sl, :].broadcast_to([128, H, 128]))
                    ind = tind[:, 128 * sub:128 * sub + 128]
                    nc.tensor.matmul(G_ps[:], ind, gme[:],
                                     start=(sub == 0), stop=(sub == nsub - 1))
                    nc.tensor.matmul(S_ps[:], ind, exts[:, sl, 0:4],
                                     start=(sub == 0), stop=(sub == nsub - 1))
                    sub += 1

            # ---- window epilogue: per-node layer 2 + softmax normalize ----
            Gsb = wk.tile([128, 512], bf, tag="Gsb")
            nc.vector.tensor_copy(Gsb[:], G_ps[:])
            GTp = ps_ax.tile([128, 512], bf, tag="ax")
            for h in range(H):
                nc.tensor.transpose(GTp[:, 128 * h:128 * h + 128],
                                    Gsb[:, 128 * h:128 * h + 128], c_ident[:])
            GTs = wk.tile([128, 512], bf, tag="GTs")
            nc.vector.tensor_copy(GTs[:], GTp[:])
            rec = wk.tile([128, 4], f32, tag="rec")
            nc.vector.reciprocal(rec[:], S_ps[:])
            nc.vector.tensor_scalar_mul(rec[:], rec[:], 0.25)
            Tp = ps_ax.tile([128, 256], f32, tag="ax")
            for h in range(H):
                nc.tensor.matmul(Tp[:, 64 * h:64 * h + 64],
                                 GTs[:, 128 * h:128 * h + 128],
                                 c_wm2t[:, 64 * h:64 * h + 64])
            tmp = wk.tile([128, 256], f32, tag="tmp")
            nc.vector.tensor_mul(tmp[:].rearrange("p (h c) -> p h c", h=H),
                                 Tp[:].rearrange("p (h c) -> p h c", h=H),
                                 rec[:].broadcast_to([128, H, 64]))
            o3 = wk.tile([128, 64], f32, tag="oc")
            nc.vector.reduce_sum(o3[:], tmp[:].rearrange("p (h c) -> p c h", h=H),
                                 axis=mybir.AxisListType.X)
            o4 = wk.tile([128, 64], f32, tag="od")
            nc.vector.tensor_add(o4[:], o3[:], c_bm2m[:])
            nc.sync.dma_start(out[128 * w:128 * w + 128, :], o4[:])

    nc.compile()
    return nc


_BUILD_CACHE = {}


def kernel(**inputs):
    per_core, consts, meta = _prep(**inputs)
    e_win = meta["e_win"]
    if e_win not in _BUILD_CACHE:
        _BUILD_CACHE[e_win] = _build(e_win)
    nc = _BUILD_CACHE[e_win]

    in_maps = []
    for cidx in range(N_CORES):
        m = dict(consts)
        m.update(per_core[cidx])
        in_maps.append(m)

    res = run_bass_kernel_spmd(nc, in_maps, core_ids=list(range(N_CORES)),
                               trace=TRACE)
    LAST_RESULT["exec_time_ns"] = res.exec_time_ns

    win_of, slot_of, deg = meta["win_of"], meta["slot_of"], meta["deg"]
    out = np.empty((N_NODES, C), np.float32)
    rows = (win_of % WPC) * 128 + slot_of
    cores = win_of // WPC
    for cidx in range(N_CORES):
        sel = cores == cidx
        out[sel] = res.results[cidx]["out"][rows[sel]]
    out[deg == 0] = 0.0
    return out


# revision 16
# speedup vs baseline: 1.1887x; 1.1887x over previous
"""Trainium2 Bass kernel for CGAtNet-style GNN message passing.

Strategy (8 NeuronCores, SPMD, no collectives needed):
  - Host: assign the 10000 nodes to 80 windows of <=128 nodes (LPT-balanced by
    in-degree), 10 windows per core.  Edges are grouped by the window of their
    dst node and padded to a fixed per-window edge count E_WIN, so every core
    runs the identical compiled graph and produces output rows for a disjoint
    node set (no cross-core reduction at all).
  - Device, per window:
      * dma_gather (transposed) pulls x[dst]^T and x[src]^T columns in bf16
      * edge-MLP runs in transposed (feature-major) space on TensorE
      * M-net layer-1 per 128-edge subtile -> g_m [128e, 512] (edge-major)
      * A-net layer-1 feature-major per head + alpha = wa2 . lrelu(h1) on PE,
        exp on ScalarE, tiny PE transposes give per-edge softmax weights
      * per-node aggregation of ex-weighted hidden activations via an
        iota==dst indicator matmul accumulated in PSUM over the window
      * window epilogue applies M-net layer-2 per *node* (32x fewer FLOPs than
        per edge), normalizes by the softmax denominator, means over heads.
  - Softmax max-subtraction is dropped: softmax is shift-invariant and alpha
    here is O(1), so exp() is numerically safe.

Self-contained: hardcodes all shapes from the problem spec.
"""

import sys

for _p in ("/opt/trn_rl_repo",):
    if _p not in sys.path:
        sys.path.append(_p)

from contextlib import ExitStack

import ml_dtypes
import numpy as np

import concourse.bacc as bacc
import concourse.bass as bass
import concourse.mybir as mybir
import concourse.tile as tile
from concourse.bass_utils import run_bass_kernel_spmd
from concourse.library_config import mlp as _mlp_lib

BF16 = ml_dtypes.bfloat16

N_NODES = 10000
N_EDGES = 320000
C = 64
H = 4
HID = 128
N_CORES = 8
WPC = 10                 # windows per core
NW = N_CORES * WPC       # 80 windows
WIN = 128                # nodes per window

AF = mybir.ActivationFunctionType
ALU = mybir.AluOpType
DT = mybir.dt

TRACE = False            # set True by test.py to capture a profile
LAST_RESULT = {}         # filled with exec_time_ns etc. for test.py


# --------------------------------------------------------------------------
# host-side preparation
# --------------------------------------------------------------------------

def _assign_windows(deg):
    """LPT-balance nodes into NW windows of <=WIN nodes each."""
    import heapq

    order = np.argsort(-deg, kind="stable")
    heap = [(0, wi) for wi in range(NW)]
    heapq.heapify(heap)
    nnodes = np.zeros(NW, np.int64)
    ecount = np.zeros(NW, np.int64)
    win_of = np.empty(N_NODES, np.int64)
    slot_of = np.empty(N_NODES, np.int64)
    for n in order:
        while True:
            e, wi = heapq.heappop(heap)
            if nnodes[wi] < WIN:
                break
        win_of[n] = wi
        slot_of[n] = nnodes[wi]
        nnodes[wi] += 1
        ecount[wi] += deg[n]
        if nnodes[wi] < WIN:
            heapq.heappush(heap, (int(ecount[wi]), wi))
    return win_of, slot_of, ecount


def _wrap_idx(a):
    """dma_gather(transpose=True) emits out col c = in[idxw[c%16, c//16]];
    column-major wrap makes output columns follow logical index order."""
    return np.tile(a.reshape(-1, 16).T, (8, 1))


def _prep(x, edge_index, edge_attr, We1, be1, We2, be2,
          Wa1, ba1, Wa2, ba2, Wm1, bm1, Wm2, bm2):
    src = np.asarray(edge_index[0], np.int64)
    dst = np.asarray(edge_index[1], np.int64)
    x = np.asarray(x, np.float32)
    edge_attr = np.asarray(edge_attr, np.float32)

    deg = np.bincount(dst, minlength=N_NODES)
    win_of, slot_of, ecount = _assign_windows(deg)
    e_win = int(ecount.max())
    e_win = ((e_win + 127) // 128) * 128
    e_win = max(e_win, 512)

    w_of_edge = win_of[dst]
    eorder = np.argsort(w_of_edge, kind="stable")
    wcounts = np.bincount(w_of_edge, minlength=NW)
    woff = np.concatenate([[0], np.cumsum(wcounts)])

    src_p = np.zeros((NW, e_win), np.int16)
    dst_p = np.zeros((NW, e_win), np.int16)
    dw_p = np.full((NW, e_win), 255.0, np.float32)
    ea_p = np.zeros((NW, e_win, C), np.float32)
    for wi in range(NW):
        sl = eorder[woff[wi]:woff[wi + 1]]
        c = len(sl)
        src_p[wi, :c] = src[sl]
        dst_p[wi, :c] = dst[sl]
        dw_p[wi, :c] = slot_of[dst[sl]]
        ea_p[wi, :c] = edge_attr[sl]

    nsub = e_win // 128
    icols = e_win // 16

    # window-node id table (slot order), padded with 0
    node_tab = np.zeros((NW, 128), np.int16)
    node_tab[win_of, slot_of] = np.arange(N_NODES, dtype=np.int16)

    ar128 = np.arange(128, dtype=np.float32)
    per_core = []
    for cidx in range(N_CORES):
        ws = range(WPC * cidx, WPC * cidx + WPC)
        eaT = np.concatenate([ea_p[w].T for w in ws], axis=1).astype(BF16)
        gidx = np.concatenate(
            [blk for w in ws for blk in (_wrap_idx(src_p[w]), _wrap_idx(node_tab[w]))],
            axis=1)
        indt_blocks = []
        indn_blocks = []
        for w in ws:
            dw = dw_p[w].reshape(nsub, 128)
            indt_blocks.append(
                (dw[:, :, None] == ar128).transpose(1, 0, 2).reshape(128, nsub * 128))
            indn_blocks.append(ar128[:, None] == dw.reshape(1, nsub * 128))
        indt = np.concatenate(indt_blocks, axis=1).astype(BF16)
        indn = np.concatenate(indn_blocks, axis=1).astype(BF16)
        per_core.append({"eaT": eaT, "gidx": gidx, "indt": indt, "indn": indn})

    xpad = np.zeros((N_NODES, 128), BF16)
    xpad[:, :C] = x.astype(BF16)
    xpad[:, C:] = x.astype(BF16)

    # weights / constants (replicated across cores)
    Wa1 = np.asarray(Wa1, np.float32)
    Wm1 = np.asarray(Wm1, np.float32)
    assert np.allclose(np.asarray(ba1), 0) and np.allclose(np.asarray(bm1), 0), \
        "nonzero layer-1 MH biases unsupported by this kernel build"
    consts = {
        "xpad": xpad,
        "wtopm": Wm1[:, :, 0:128].transpose(2, 0, 1).reshape(128, 512).astype(BF16),
        "wbotm": np.concatenate([Wm1[:, :, 128:192].transpose(2, 0, 1).reshape(64, 512)] * 2,
                                axis=0).astype(BF16),
        "wa1t": Wa1[:, :, 0:128].transpose(2, 0, 1).reshape(128, 512).astype(BF16),
        "wa1b": np.concatenate([Wa1[:, :, 128:192].transpose(2, 0, 1).reshape(64, 512)] * 2,
                                axis=0).astype(BF16),
        "wa2r": np.repeat(np.asarray(Wa2, np.float32)[:, 0, :].T, 32, axis=1).astype(BF16).copy(),
        "wm2t": np.asarray(Wm2, np.float32).transpose(2, 0, 1).reshape(128, 256).astype(BF16),
        "we1t": np.asarray(We1, np.float32).T.astype(BF16).copy(),
        "we2t": np.asarray(We2, np.float32).T.astype(BF16).copy(),
        "ident": np.eye(128, dtype=np.float32).astype(BF16),
        "be1c": np.asarray(be1, np.float32).reshape(64, 1).copy(),
        "bxe": np.concatenate([np.zeros(64, np.float32),
                               np.asarray(be2, np.float32).reshape(64)]).reshape(128, 1).copy(),
        "ba2c": _ba2col(ba2),
        "bm2m": np.tile(np.asarray(bm2, np.float32).mean(axis=0), (128, 1)).copy(),
    }

    meta = {"e_win": e_win, "nsub": nsub, "icols": icols,
            "win_of": win_of, "slot_of": slot_of, "deg": deg}
    return per_core, consts, meta


def _ba2col(ba2):
    v = np.asarray(ba2, np.float32).reshape(H)
    return np.repeat(v, 32).reshape(128, 1).copy()


# --------------------------------------------------------------------------
# device graph
# --------------------------------------------------------------------------

def _build(e_win):
    nsub = e_win // 128
    icols = e_win // 16
    nc = bacc.Bacc("TRN2", target_bir_lowering=False)
    bf, f32, i16 = DT.bfloat16, DT.float32, DT.int16

    P = lambda name, shape, dt: nc.declare_dram_parameter(name, shape, dt, isOutput=False)
    xpad = P("xpad", [N_NODES, 128], bf)
    eaT = P("eaT", [64, WPC * e_win], bf)
    gidx = P("gidx", [128, WPC * (icols + 8)], i16)
    indt = P("indt", [128, WPC * nsub * 128], bf)
    indn = P("indn", [128, WPC * nsub * 128], bf)
    wtopm = P("wtopm", [128, 512], bf)
    wbotm = P("wbotm", [128, 512], bf)
    wa1t = P("wa1t", [128, 512], bf)
    wa1b = P("wa1b", [128, 512], bf)
    wa2r = P("wa2r", [128, 128], bf)
    wm2t = P("wm2t", [128, 256], bf)
    we1t = P("we1t", [64, 64], bf)
    we2t = P("we2t", [64, 64], bf)
    ident = P("ident", [128, 128], bf)
    be1c = P("be1c", [64, 1], f32)
    bxe = P("bxe", [128, 1], f32)
    ba2c = P("ba2c", [128, 1], f32)
    bm2m = P("bm2m", [128, 64], f32)
    out = nc.declare_dram_parameter("out", [WPC * 128, 64], f32, isOutput=True)

    with tile.TileContext(nc) as tc, ExitStack() as ctx:
        pool = lambda name, bufs, **kw: ctx.enter_context(
            tc.tile_pool(name=name, bufs=bufs, **kw))
        cpool = pool("consts", 1)
        gath = pool("gath", 6)
        eap = pool("eap", 2)
        ip = pool("ip", 2)
        wk = pool("work", 2)
        wk3 = pool("work3", 3)
        ps_h1 = pool("ps_h1", 2, space="PSUM")
        ps_ax = pool("ps_ax", 2, space="PSUM")
        ps_g = pool("ps_g", 1, space="PSUM")
        ps_s = pool("ps_s", 1, space="PSUM")
        ps_m = pool("ps_m", 2, space="PSUM")

        def cload(name, src_ap, shape, dt):
            t = cpool.tile(shape, dt, tag=name)
            nc.sync.dma_start(t[:], src_ap)
            return t

        c_wtopm = cload("wtopm", wtopm[:], [128, 512], bf)
        c_wbotm = cload("wbotm", wbotm[:], [128, 512], bf)
        c_wa1t = cload("wa1t", wa1t[:], [128, 512], bf)
        c_wa1b = cload("wa1b", wa1b[:], [128, 512], bf)
        c_wa2r = cload("wa2r", wa2r[:], [128, 128], bf)
        c_wm2t = cload("wm2t", wm2t[:], [128, 256], bf)
        c_we1t = cload("we1t", we1t[:], [64, 64], bf)
        c_we2t = cload("we2t", we2t[:], [64, 64], bf)
        c_ident = cload("ident", ident[:], [128, 128], bf)
        c_be1c = cload("be1c", be1c[:], [64, 1], f32)
        c_bxe = cload("bxe", bxe[:], [128, 1], f32)
        c_ba2c = cload("ba2c", ba2c[:], [128, 1], f32)
        c_bm2m = cload("bm2m", bm2m[:], [128, 64], f32)
        c_idx = cload("gidx", gidx[:], [128, WPC * (icols + 8)], i16)

        nc.gpsimd.load_library(_mlp_lib)

        for w in range(WPC):
            xi = gath.tile([128, 1, e_win], bf, tag="m12")
            xj = gath.tile([128, 1, e_win], bf, tag="gath")
            xw = gath.tile([128, 1, 128], bf, tag="xw")
            ib = w * (icols + 8)
            nc.gpsimd.dma_gather(xj[:], xpad[:], c_idx[:, ib:ib + icols],
                                 e_win, e_win, 128, transpose=True,
                                 single_packet=False)
            nc.gpsimd.dma_gather(xw[:], xpad[:], c_idx[:, ib + icols:ib + icols + 8],
                                 128, 128, 128, transpose=False)
            ea = eap.tile([64, e_win], bf)
            nc.sync.dma_start(ea[:], eaT[:, w * e_win:(w + 1) * e_win])
            tind = ip.tile([128, nsub * 128], bf, tag="indt")
            nc.sync.dma_start(tind[:], indt[:, w * nsub * 128:(w + 1) * nsub * 128])
            nind = ip.tile([128, nsub * 128], bf, tag="indn")
            nc.sync.dma_start(nind[:], indn[:, w * nsub * 128:(w + 1) * nsub * 128])

            # edge-MLP (feature-major) + selector-gathered x_i share one psum
            for c0 in range(0, e_win, 512):
                cw = min(512, e_win - c0)
                pe1 = ps_m.tile([64, 512], f32, tag="misc")
                nc.tensor.matmul(pe1[:, :cw], c_we1t[:], ea[:, c0:c0 + cw])
                te = wk.tile([64, 512], bf, tag="te")
                nc.scalar.activation(te[:, :cw], pe1[:, :cw], AF.Prelu,
                                     bias=c_be1c[:], alpha=0.01)
                xie = ps_m.tile([128, 512], f32, tag="misc")
                nc.tensor.matmul(xie[0:64, :cw], xw[:, 0, 0:64],
                                 nind[:, c0:c0 + cw], tile_position=(0, 0))
                nc.tensor.matmul(xie[64:128, :cw], c_we2t[:], te[:, :cw],
                                 tile_position=(0, 64))
                nc.scalar.activation(xi[0:128, 0, c0:c0 + cw], xie[:, :cw],
                                     AF.Identity, bias=c_bxe[:])

            G_ps = ps_g.tile([128, 512], f32)
            S_ps = ps_s.tile([128, 4], f32)
            sub = 0
            for g0 in range(0, e_win, 512):
                fg = min(512, e_win - g0)
                nsg = fg // 128
                # A-net (feature-major) + alpha on PE
                aps = ps_ax.tile([128, 512], f32, tag="ax")
                for h0 in range(0, H, 2):
                    gaps0 = ps_h1.tile([128, 512], f32, tag="h1")
                    gaps1 = ps_h1.tile([128, 512], f32, tag="h1")
                    nc.tensor.matmul(gaps0[:, :fg], c_wa1t[:, 128 * h0:128 * h0 + 128],
                                     xi[0:128, 0, g0:g0 + fg], start=True, stop=False)
                    nc.tensor.matmul(gaps1[:, :fg], c_wa1t[:, 128 * h0 + 128:128 * h0 + 256],
                                     xi[0:128, 0, g0:g0 + fg], start=True, stop=False)
                    nc.tensor.matmul(gaps0[:, :fg], c_wa1b[0:64, 128 * h0:128 * h0 + 128],
                                     xj[0:64, 0, g0:g0 + fg], start=False, stop=True,
                                     tile_position=(0, 0))
                    nc.tensor.matmul(gaps1[:, :fg], c_wa1b[64:128, 128 * h0 + 128:128 * h0 + 256],
                                     xj[64:128, 0, g0:g0 + fg], start=False, stop=True,
                                     tile_position=(64, 0))
                    for dh in range(2):
                        h = h0 + dh
                        gaps = gaps0 if dh == 0 else gaps1
                        ga = wk.tile([128, 512], bf, tag="ga")
                        nc.scalar.activation(ga[:, :fg], gaps[:, :fg], AF.Prelu, alpha=0.01)
                        nc.tensor.matmul(aps[32 * h:32 * h + 32, :fg],
                                         c_wa2r[:, 32 * h:32 * h + 32], ga[:, :fg],
                                         tile_position=(0, 32 * h))
                # row 32h..32h+32 of aps all hold head h's alpha (wa2 replicated)
                exsb = wk.tile([128, 512], bf, tag="ex")
                nc.scalar.activation(exsb[:, :fg], aps[:, :fg], AF.Exp,
                                     bias=c_ba2c[:])
                extp = ps_ax.tile([128, 4, 128], bf, tag="ax")
                for sl in range(nsg):
                    nc.tensor.transpose(extp[:, sl, :],
                                        exsb[:, 128 * sl:128 * sl + 128],
                                        c_ident[:])
                exts = wk.tile([128, 4, 4], bf, tag="exts")
                nc.vector.tensor_copy(exts[:], extp[:, :, 0:128:32])

                # M-net subtiles + indicator aggregation
                for sl0 in range(0, nsg, 2):
                  pair = [sl0] + ([sl0 + 1] if sl0 + 1 < nsg else [])
                  h1ms = []
                  for k, sl in enumerate(pair):
                    e0 = g0 + 128 * sl
                    h1m = ps_h1.tile([128, 512], f32, tag="h1")
                    h1ms.append(h1m)
                    nc.tensor.matmul(h1m[:], xi[0:128, 0, e0:e0 + 128], c_wtopm[:],
                                     start=True, stop=False)
                  for k, sl in enumerate(pair):
                    e0 = g0 + 128 * sl
                    r0 = 64 * k
                    nc.tensor.matmul(h1ms[k][:], xj[r0:r0 + 64, 0, e0:e0 + 128],
                                     c_wbotm[r0:r0 + 64, :],
                                     start=False, stop=True, tile_position=(r0, 0))
                  for k, sl in enumerate(pair):
                    h1m = h1ms[k]
                    gm = wk3.tile([128, 512], bf, tag="gm")
                    nc.scalar.activation(gm[:], h1m[:], AF.Prelu, alpha=0.01)
                    gme = wk3.tile([128, 512], bf, tag="gme")
                    nc.vector.tensor_mul(
                        gme[:].rearrange("p (h j) -> p h j", h=H),
                        gm[:].rearrange("p (h j) -> p h j", h=H),
                        exts[:, sl, :].broadcast_to([128, H, 128]))
                    ind = tind[:, 128 * sub:128 * sub + 128]
                    nc.tensor.matmul(G_ps[:], ind, gme[:],
                                     start=(sub == 0), stop=(sub == nsub - 1))
                    nc.tensor.matmul(S_ps[:], ind, exts[:, sl, 0:4],
                                     start=(sub == 0), stop=(sub == nsub - 1))
                    sub += 1

            # ---- window epilogue: per-node layer 2 + softmax normalize ----
            Gsb = wk.tile([128, 512], bf, tag="Gsb")
            nc.vector.tensor_copy(Gsb[:], G_ps[:])
            GTp = ps_ax.tile([128, 512], bf, tag="ax")
            for h in range(H):
                nc.tensor.transpose(GTp[:, 128 * h:128 * h + 128],
                                    Gsb[:, 128 * h:128 * h + 128], c_ident[:])
            GTs = wk.tile([128, 512], bf, tag="GTs")
            nc.vector.tensor_copy(GTs[:], GTp[:])
            rec = wk.tile([128, 4], f32, tag="rec")
            nc.vector.reciprocal(rec[:], S_ps[:])
            nc.vector.tensor_scalar_mul(rec[:], rec[:], 0.25)
            Tp = ps_ax.tile([128, 256], f32, tag="ax")
            for h in range(H):
                nc.tensor.matmul(Tp[:, 64 * h:64 * h + 64],
                                 GTs[:, 128 * h:128 * h + 128],
                                 c_wm2t[:, 64 * h:64 * h + 64])
            tmp = wk.tile([128, 256], f32, tag="tmp")
            nc.vector.tensor_mul(tmp[:].rearrange("p (h c) -> p h c", h=H),
                                 Tp[:].rearrange("p (h c) -> p h c", h=H),
                                 rec[:].broadcast_to([128, H, 64]))
            o3 = wk.tile([128, 64], f32, tag="oc")
            nc.vector.reduce_sum(o3[:], tmp[:].rearrange("p (h c) -> p c h", h=H),
                                 axis=mybir.AxisListType.X)
            o4 = wk.tile([128, 64], f32, tag="od")
            nc.vector.tensor_add(o4[:], o3[:], c_bm2m[:])
            nc.sync.dma_start(out[128 * w:128 * w + 128, :], o4[:])

    nc.compile()
    return nc


_BUILD_CACHE = {}


def kernel(**inputs):
    per_core, consts, meta = _prep(**inputs)
    e_win = meta["e_win"]
    if e_win not in _BUILD_CACHE:
        _BUILD_CACHE[e_win] = _build(e_win)
    nc = _BUILD_CACHE[e_win]

    in_maps = []
    for cidx in range(N_CORES):
        m = dict(consts)
        m.update(per_core[cidx])
        in_maps.append(m)

    res = run_bass_kernel_spmd(nc, in_maps, core_ids=list(range(N_CORES)),
                               trace=TRACE)
    LAST_RESULT["exec_time_ns"] = res.exec_time_ns

    win_of, slot_of, deg = meta["win_of"], meta["slot_of"], meta["deg"]
    out = np.empty((N_NODES, C), np.float32)
    rows = (win_of % WPC) * 128 + slot_of
    cores = win_of // WPC
    for cidx in range(N_CORES):
        sel = cores == cidx
        out[sel] = res.results[cidx]["out"][rows[sel]]
    out[deg == 0] = 0.0
    return out
